# revision 15
# baseline (speedup 1.0000x reference)
"""Trainium2 Bass kernel for nn_MultiHeadAttention_55250459296450.

Multi-head attention with relative-position embeddings (torch view-semantics
scramble preserved), causal mask, B=8 S=1024 E=1024 H=16 D=64, L=1024.

Sharding: data-parallel over batch: core c computes the full output for
batch c; the scrambled RPE-score q slices (heads {c, c+8} of every batch)
are re-projected locally, so no collectives are needed.

Per-core pipeline:
  1. QT/KT (e-major) + V (s-major) projections (fp32r), Q_rpeT (bf16).
  2. Per head: G = q_rpe @ TkT -> bf16 DRAM buffer whose 512 pad columns
     hold -1e9; score psum = q@kT + skewed-G via identity-matmul accumulate
     (skewed view row-stride 1535 turns G[i, j-i+1023] into a strided DMA,
     and pad-column overruns implement the causal mask); exp on ACT with
     accum_out row sums; normalize; attn bf16 -> DRAM E buffer; attn^T and
     skewed-attn^T tiles return via DMA-transpose (XBAR, row strides
     1024/1025); ghost elements masked; AV + Tv-table matmuls -> outcT.
  3. out = outcT^T @ Wo^T (bf16).
"""

import numpy as np
import ml_dtypes

B, S, E, H = 8, 1024, 1024, 16
D = E // H
P = 128
NB = S // P
GW = 1536
EW = 1024
E_OFF = 1023
E_SIZE = 17 * P * 512
NEG = -1.0e9

_cache = {}


def _build():
    import concourse.bass as bass
    import concourse.tile as tile
    import concourse.mybir as mybir
    from concourse import bacc
    from contextlib import ExitStack

    dt = mybir.dt
    f32, bf16, f32r = dt.float32, dt.bfloat16, dt.float32r
    AF = mybir.ActivationFunctionType

    nc = bacc.Bacc("TRN2", target_bir_lowering=False, debug=False,
                   enable_asserts=False, num_devices=1)

    t_xo = nc.dram_tensor("xT_own", [S, S], f32r, kind="ExternalInput")
    t_xa = nc.dram_tensor("xT_all", [B, S, S], bf16, kind="ExternalInput")
    t_wq = nc.dram_tensor("WqT", [E, E], f32r, kind="ExternalInput")
    t_wk = nc.dram_tensor("WkT", [E, E], f32r, kind="ExternalInput")
    t_wv = nc.dram_tensor("WvT", [E, E], f32r, kind="ExternalInput")
    t_wqr = nc.dram_tensor("WqTr", [E, 2 * D], bf16, kind="ExternalInput")
    t_wo = nc.dram_tensor("WoT", [E, E], bf16, kind="ExternalInput")
    t_tk = nc.dram_tensor("TkT", [P, S], bf16, kind="ExternalInput")
    t_tv = nc.dram_tensor("TvS", [S, D], bf16, kind="ExternalInput")
    t_skm = nc.dram_tensor("skmask", [P, S], bf16, kind="ExternalInput")
    t_id = nc.dram_tensor("ident", [P, P], bf16, kind="ExternalInput")
    t_neg = nc.dram_tensor("negpad", [P, 512], bf16, kind="ExternalInput")
    t_out = nc.dram_tensor("out_c", [S, E], f32, kind="ExternalOutput")

    def copy_alt(i, out, in_):
        if i % 2 == 0:
            nc.scalar.copy(out, in_)
        else:
            nc.vector.tensor_copy(out, in_)

    def skew_read_ap(gbuf, k, jt):
        off = (P * k) * (GW - 1) + 512 * jt + 1023
        return bass.AP(tensor=gbuf.tensor, offset=gbuf.offset + off,
                       ap=[[GW - 1, P], [1, 512]])

    def e_write_ap(ebuf, k, w):
        off = E_OFF + (P * k) * EW
        return bass.AP(tensor=ebuf.tensor, offset=ebuf.offset + off,
                       ap=[[EW, P], [1, w]])

    def at_src_ap(ebuf, m):
        rows = S - P * m
        off = E_OFF + (P * m) * EW + P * m
        return bass.AP(tensor=ebuf.tensor, offset=ebuf.offset + off,
                       ap=[[EW, rows], [1, P]])

    def a2t_src_ap(ebuf, n):
        rows = P * (n + 1)
        i_lo = P * (NB - 1 - n)
        off = i_lo * (EW + 1) + P * n
        return bass.AP(tensor=ebuf.tensor, offset=ebuf.offset + off,
                       ap=[[EW + 1, rows], [1, P]])

    with tile.TileContext(nc) as tc, ExitStack() as ctx:
        pers = ctx.enter_context(tc.tile_pool(name="pers", bufs=1))
        qt = pers.tile([P, NB, S], bf16, tag="qt")
        kt = pers.tile([P, NB, S], bf16, tag="kt")
        vv = pers.tile([P, NB, E], bf16, tag="vv")
        qr = pers.tile([P, B, S], bf16, tag="qr")
        oc = pers.tile([P, NB, S], bf16, tag="oc")
        tk = pers.tile([P, S], bf16, tag="tk")
        tv = pers.tile([P, NB, D], bf16, tag="tv")
        skm = pers.tile([P, S], bf16, tag="skm")
        ident = pers.tile([P, P], bf16, tag="ident")
        negt = pers.tile([P, 512], bf16, tag="negt")

        nc.sync.dma_start(tk[:], t_tk.ap())
        nc.sync.dma_start(tv[:], t_tv.ap().rearrange("(ub p) d -> p ub d", p=P))
        nc.sync.dma_start(skm[:], t_skm.ap())
        nc.sync.dma_start(ident[:], t_id.ap())
        nc.sync.dma_start(negt[:], t_neg.ap())

        dpool = ctx.enter_context(tc.tile_pool(name="dram", bufs=1, space="DRAM"))
        gbufs = [dpool.tile([S, GW], bf16, tag=f"g{i}", name=f"gbuf{i}")
                 for i in range(2)]
        ebufs = [dpool.tile([E_SIZE], bf16, tag=f"e{i}", name=f"ebuf{i}")
                 for i in range(2)]
        zt = pers.tile([P, 512], bf16, tag="zt")
        nc.vector.memset(zt[:], 0.0)
        for gb in gbufs:
            for k in range(NB):
                nc.sync.dma_start(gb[P * k:P * (k + 1), 1024:1536], negt[:])
        for ebf in ebufs:
            for ch in range(17):
                dst = bass.AP(tensor=ebf.tensor, offset=ebf.offset + ch * P * 512,
                              ap=[[512, P], [1, 512]])
                nc.sync.dma_start(dst, zt[:])

        psS = ctx.enter_context(tc.tile_pool(name="psS", bufs=2, space="PSUM"))
        psG = ctx.enter_context(tc.tile_pool(name="psG", bufs=2, space="PSUM"))
        psA = ctx.enter_context(tc.tile_pool(name="psA", bufs=2, space="PSUM"))

        with tc.tile_pool(name="projA", bufs=1) as pA:
            xo = pA.tile([P, NB, S], f32r, tag="xo")
            nc.sync.dma_start(xo[:], t_xo.ap().rearrange("(fb p) s -> p fb s", p=P))
            for name, t_w, dst, emaj in (("q", t_wq, qt, True),
                                         ("k", t_wk, kt, True),
                                         ("v", t_wv, vv, False)):
                w = pA.tile([P, NB, E], f32r, tag="w", name="w")
                nc.sync.dma_start(w[:], t_w.ap().rearrange("(fb p) e -> p fb e", p=P))
                if emaj:
                    for eb in range(NB):
                        for sh in range(2):
                            ps = psS.tile([P, 1024], f32, tag="s", name="ps")
                            for fb in range(NB):
                                nc.tensor.matmul(
                                    ps[:, :512],
                                    w[:, fb, P * eb:P * (eb + 1)],
                                    xo[:, fb, 512 * sh:512 * (sh + 1)],
                                    start=(fb == 0), stop=(fb == NB - 1))
                            copy_alt(eb + sh, dst[:, eb, 512 * sh:512 * (sh + 1)],
                                     ps[:, :512])
                else:
                    for sb in range(NB):
                        for eh in range(2):
                            ps = psS.tile([P, 1024], f32, tag="s", name="ps")
                            for fb in range(NB):
                                nc.tensor.matmul(
                                    ps[:, :512],
                                    xo[:, fb, P * sb:P * (sb + 1)],
                                    w[:, fb, 512 * eh:512 * (eh + 1)],
                                    start=(fb == 0), stop=(fb == NB - 1))
                            copy_alt(sb + eh, dst[:, sb, 512 * eh:512 * (eh + 1)],
                                     ps[:, :512])

        with tc.tile_pool(name="projB", bufs=1) as pB, \
             tc.tile_pool(name="projBx", bufs=2) as pBx:
            wqr = pB.tile([P, NB, 2 * D], bf16, tag="wqr")
            nc.sync.dma_start(wqr[:], t_wqr.ap().rearrange("(fb p) e -> p fb e", p=P))
            for b in range(B):
                xb = pBx.tile([P, NB, S], bf16, tag="xb", name="xb")
                nc.sync.dma_start(
                    xb[:], t_xa.ap()[b].rearrange("(fb p) s -> p fb s", p=P))
                for sh in range(2):
                    ps = psS.tile([P, 1024], f32, tag="s", name="ps")
                    for fb in range(NB):
                        nc.tensor.matmul(ps[:, :512], wqr[:, fb, :],
                                         xb[:, fb, 512 * sh:512 * (sh + 1)],
                                         start=(fb == 0), stop=(fb == NB - 1))
                    copy_alt(b + sh, qr[:, b, 512 * sh:512 * (sh + 1)], ps[:, :512])

        hp_pool = ctx.enter_context(tc.tile_pool(name="hwork", bufs=3))
        zt_pool = ctx.enter_context(tc.tile_pool(name="zwork", bufs=6))
        at_pool = ctx.enter_context(tc.tile_pool(name="atwork", bufs=11))
        for hp in range(H):
            b = hp // 2
            hc = 64 * (hp % 2)
            eb = hp // 2
            gb = gbufs[hp % 2]
            ebuf = ebufs[hp % 2]
            q_l = qt[hc:hc + D, eb]
            k_l = kt[hc:hc + D, eb]
            qr_l = qr[hc:hc + D, b]

            for k in range(NB):
                w = P * (k + 1)
                u0 = S - w
                for gm in range((w + 511) // 512):
                    cols = min(512, w - 512 * gm)
                    psg = psG.tile([P, 512], f32, tag="g", name="psg")
                    nc.tensor.matmul(
                        psg[:, :cols],
                        qr_l[:, P * k:P * (k + 1)],
                        tk[hc:hc + D, u0 + 512 * gm:u0 + 512 * gm + cols],
                        start=True, stop=True)
                    gsb = hp_pool.tile([P, 512], bf16, tag="gsb", name="gsb")
                    copy_alt(k + gm, gsb[:, :cols], psg[:, :cols])
                    nc.sync.dma_start(
                        gb[P * k:P * (k + 1), u0 + 512 * gm:u0 + 512 * gm + cols],
                        gsb[:, :cols])

            for k in range(NB):
                nj = 1 if k < 4 else 2
                jw = 512 * nj
                ew = P * (k + 1)
                ps = psS.tile([P, 1024], f32, tag="s", name="ps")
                for jt in range(nj):
                    nc.tensor.matmul(ps[:, 512 * jt:512 * (jt + 1)],
                                     q_l[:, P * k:P * (k + 1)],
                                     k_l[:, 512 * jt:512 * (jt + 1)],
                                     start=True, stop=False,
                                     skip_group_check=True)
                    gsk = hp_pool.tile([P, 512], bf16, tag="gsk", name="gsk")
                    nc.sync.dma_start(gsk[:], skew_read_ap(gb, k, jt))
                    nc.tensor.matmul(ps[:, 512 * jt:512 * (jt + 1)],
                                     ident[:], gsk[:],
                                     start=False, stop=(jt == nj - 1),
                                     skip_group_check=True)
                esb = hp_pool.tile([P, 1024], bf16, tag="esb", name="esb")
                zr = zt_pool.tile([P, 1], f32, tag="zr", name="zr")
                nc.scalar.activation(esb[:, :jw], ps[:, :jw], AF.Exp,
                                     scale=0.125, accum_out=zr[:])
                rz = zt_pool.tile([P, 1], f32, tag="rz", name="rz")
                nc.vector.reciprocal(rz[:], zr[:])
                nc.vector.tensor_scalar_mul(esb[:, :ew], esb[:, :ew], rz[:])
                nc.sync.dma_start(e_write_ap(ebuf, k, ew), esb[:, :ew])

            ats = []
            for m in range(NB):
                rows = S - P * m
                at = at_pool.tile([P, S], bf16, tag="at", name="at")
                nc.sync.dma_start_transpose(at[:, :rows], at_src_ap(ebuf, m))
                ats.append(at)
            a2s = []
            for n in range(NB):
                rows = P * (n + 1)
                a2 = at_pool.tile([P, S], bf16, tag="a2", name="a2")
                nc.sync.dma_start_transpose(a2[:, :rows], a2t_src_ap(ebuf, n))
                nc.vector.tensor_mul(a2[:, :rows], a2[:, :rows], skm[:, :rows])
                a2s.append(a2)
            for it in range(2):
                pav = psA.tile([D, 512], f32, tag="av", name="pav")
                for m in range(4 * it + 4):
                    si = max(512 * it, P * m)
                    n_cols = 512 * (it + 1) - si
                    nc.tensor.matmul(pav[:, si - 512 * it:512],
                                     vv[:, m, 64 * hp:64 * hp + D],
                                     ats[m][:, si - P * m:si - P * m + n_cols],
                                     start=(m == 0), stop=False,
                                     skip_group_check=True)
                for n in range(4 - 4 * it, NB):
                    i_lo = P * (NB - 1 - n)
                    si = max(512 * it, i_lo)
                    n_cols = 512 * (it + 1) - si
                    nc.tensor.matmul(pav[:, si - 512 * it:512],
                                     tv[:, n, :],
                                     a2s[n][:, si - i_lo:si - i_lo + n_cols],
                                     start=False, stop=(n == NB - 1),
                                     skip_group_check=True)
                copy_alt(hp + it, oc[hc:hc + D, eb, 512 * it:512 * (it + 1)],
                         pav[:])

        with tc.tile_pool(name="projO", bufs=1) as pO, \
             tc.tile_pool(name="projOo", bufs=3) as pOo:
            wo = pO.tile([P, NB, E], bf16, tag="wo")
            nc.sync.dma_start(wo[:], t_wo.ap().rearrange("(eb p) eo -> p eb eo", p=P))
            for sb in range(NB):
                for eo in range(2):
                    ps = psS.tile([P, 1024], f32, tag="s", name="ps")
                    for ebo in range(NB):
                        nc.tensor.matmul(ps[:, :512],
                                         oc[:, ebo, P * sb:P * (sb + 1)],
                                         wo[:, ebo, 512 * eo:512 * (eo + 1)],
                                         start=(ebo == 0), stop=(ebo == NB - 1))
                    osb = pOo.tile([P, 512], f32, tag="osb", name="osb")
                    copy_alt(sb + eo, osb[:], ps[:, :512])
                    nc.sync.dma_start(
                        t_out.ap()[P * sb:P * (sb + 1), 512 * eo:512 * (eo + 1)],
                        osb[:])

    nc.compile()
    return nc


def _prep_inputs(x, Wq, Wk, Wv, Wo, pek_table, pev_table):
    bf = ml_dtypes.bfloat16
    xT = np.ascontiguousarray(x.transpose(0, 2, 1))
    xT_all = xT.astype(bf)
    WqT = np.ascontiguousarray(Wq.T)
    WkT = np.ascontiguousarray(Wk.T)
    WvT = np.ascontiguousarray(Wv.T)
    WoT = np.ascontiguousarray(Wo.T).astype(bf)
    TkT = np.ascontiguousarray(pek_table[1:1025].T).astype(bf)
    TkT2 = np.vstack([TkT, TkT])
    TvS = np.ascontiguousarray(pev_table[1:1025]).astype(bf)
    pp, xx = np.arange(P)[:, None], np.arange(S)[None, :]
    skmask = ((pp + xx) >= (P - 1)).astype(bf)
    ident = np.eye(P, dtype=bf)
    negpad = np.full((P, 512), NEG, dtype=bf)

    in_maps = []
    for c in range(B):
        cols = np.r_[c * D:(c + 1) * D, (c + 8) * D:(c + 9) * D]
        in_maps.append({
            "xT_own": np.ascontiguousarray(xT[c]),
            "xT_all": xT_all,
            "WqT": WqT, "WkT": WkT, "WvT": WvT,
            "WqTr": np.ascontiguousarray(WqT[:, cols]).astype(bf),
            "WoT": WoT, "TkT": TkT2, "TvS": TvS,
            "skmask": skmask, "ident": ident, "negpad": negpad,
        })
    return in_maps


def kernel(x, mask, Wq, Wk, Wv, Wo, pek_table, pev_table):
    from concourse.bass_utils import run_bass_kernel_spmd

    if "nc" not in _cache:
        _cache["nc"] = _build()
    nc = _cache["nc"]
    in_maps = _prep_inputs(x, Wq, Wk, Wv, Wo, pek_table, pev_table)
    res = run_bass_kernel_spmd(nc, in_maps, core_ids=list(range(B)))
    return np.stack([res.results[c]["out_c"] for c in range(B)], axis=0)
